# revision 8
# baseline (speedup 1.0000x reference)
"""TransformerConv (heads=1) + ELU layer as a Bass/Tile kernel on 8 NeuronCores.

Strategy (1D graph partition by target node):
  - dst nodes sharded 8 ways (12500/core, padded to 98 blocks x 128).
  - Per core, nodes are re-ranked by local src-degree so every referenced
    src (~63.2k of 100k) lands in rank < 65536.  Each core computes
    kv rows [v(128)|k(128)] (bf16, 512B) for the top-65536 ranked nodes into
    DRAM kv_tab (the host pre-permutes x columns by rank), plus
    q'=(x@Wq+bq)/sqrt(d) into DRAM q_tab in dst device-row order and
    skip=x@Ws+(bs+bv) kept in SBUF.  The k bias cancels inside the per-dst
    segment softmax; the v bias sums to bv (sum alpha = 1) and is folded
    into the skip bias.
  - Edges are bucketed by dst block, then split per block into class A
    (src rank < 32768) and class B (rank in [32768, 65536)) chunk groups so
    int16 dma_gather indices can address the kv table.  Per GROUP of 2
    blocks the core issues ONE dma_gather per class for kv rows and ONE for
    q' rows (SWDGE fixed cost ~1us amortized over ~10-20 chunks), computes
    logits = rowsum(qg*kg) with wide DVE mult+reduce, ex = exp(logit),
    overwrites k[0] with 1.0 (den fold), then per 128-edge chunk builds the
    ex-weighted one-hot M[e,d] = ex_e * (dstloc_e == d) in one fused
    tensor_scalar and scatter-adds on the PE: pagg[:,0:129] += M^T @ [Vg|1].
  - Epilogue per block: out = elu(agg/den + skip), streamed to DRAM.
Pad slots gather row 0 (real data) and have dstloc=255 so their one-hot row
is zero - they contribute nothing.
"""
import math
import numpy as np
import ml_dtypes

BF16 = ml_dtypes.bfloat16

N, E, D = 100000, 800000, 128
M_CORES = 8
DPC = N // M_CORES                 # 12500
NB = (DPC + 127) // 128            # 98
DST_PAD = NB * 128                 # 12544
NREF = 65536                       # kv table rows (2 int16 classes)
HALF = 32768
SCALE = 1.0 / math.sqrt(D)
TW = 2048                          # phase-1 row-tile width
GB = 2                             # blocks per gather group


def _wrap16(cols):
    """[128, n] chunk-slot layout -> dma_gather int16 index layout [128, n*8].

    Slot (p, chunk c) sits at flat position c*128+p; dma_gather reads flat i
    from partition i%16, column i//16, replicated across the 8 groups of 16
    partitions.
    """
    npart, ncol = cols.shape
    assert npart == 128
    out = np.zeros((128, ncol * 8), np.int16)
    flat = cols.T.reshape(-1)                      # c-major, p-minor
    w = flat.reshape(-1, 16).T                     # [16, n*8]
    for g in range(8):
        out[g * 16:(g + 1) * 16] = w
    return out


def _host_prep(edge_index):
    """Rank nodes per core, pack edges into per-(block, class) chunks.

    Returns (plans, profile) where profile = ((cA, cB) x NB) is shared by all
    cores and plans[c] holds idx16_kv, idx16_q, dstloc, node_rank, perm.
    """
    src = np.asarray(edge_index[0], dtype=np.int64)
    dst = np.asarray(edge_index[1], dtype=np.int64)
    core = dst // DPC
    ld = dst - core * DPC

    cores = []
    for c in range(M_CORES):
        sel = core == c
        e_ld = ld[sel]
        e_src = src[sel]
        # per-core src-degree ranking
        sdeg = np.bincount(e_src, minlength=N)
        rank_of = np.empty(N, np.int64)
        order = np.argsort(-sdeg, kind="stable")
        rank_of[order] = np.arange(N)
        nref = int((sdeg > 0).sum())
        if nref > NREF:
            raise RuntimeError(f"core {c}: {nref} referenced srcs > {NREF}")
        e_rank = rank_of[e_src]

        # dst -> block assignment (LPT on total edges, 98 bins)
        deg = np.bincount(e_ld, minlength=DST_PAD)[:DST_PAD]
        dorder = np.argsort(-deg, kind="stable")
        loads = np.zeros(NB, np.int64)
        assign = np.zeros(DST_PAD, np.int64)
        for k in range(128):
            batch = dorder[k * NB:(k + 1) * NB]
            binord = np.argsort(loads, kind="stable")
            assign[batch] = binord
            loads[binord] += deg[batch]

        # per-block per-class counts
        e_blk = assign[e_ld]
        e_cls = (e_rank >= HALF).astype(np.int64)   # 0 = A, 1 = B
        nA = np.bincount(e_blk[e_cls == 0], minlength=NB)
        nB_ = np.bincount(e_blk[e_cls == 1], minlength=NB)
        cA = (nA + 127) // 128
        cB = (nB_ + 127) // 128
        cores.append(dict(e_ld=e_ld, e_rank=e_rank, e_blk=e_blk, e_cls=e_cls,
                          assign=assign, cA=cA, cB=cB, order=order))

    # shared profile: per core sort blocks by (cA+cB, cA) desc, take
    # coordinate-wise max at each position
    sorted_idx = []
    for c in range(M_CORES):
        key = cores[c]["cA"] * 1000 + cores[c]["cB"] + (cores[c]["cA"] + cores[c]["cB"]) * 10 ** 6
        si = np.argsort(-key, kind="stable")
        sorted_idx.append(si)
    profA = np.zeros(NB, np.int64)
    profB = np.zeros(NB, np.int64)
    for i in range(NB):
        for c in range(M_CORES):
            b = sorted_idx[c][i]
            profA[i] = max(profA[i], cores[c]["cA"][b])
            profB[i] = max(profB[i], cores[c]["cB"][b])
    profile = tuple((int(a), int(b)) for a, b in zip(profA, profB))

    # global chunk column layout
    groups = []
    b0 = 0
    while b0 < NB:
        groups.append(tuple(range(b0, min(b0 + GB, NB))))
        b0 += GB
    # per block: (A chunk col start, B chunk col start)
    colA = np.zeros(NB, np.int64)
    colB = np.zeros(NB, np.int64)
    col = 0
    for g in groups:
        for b in g:
            colA[b] = col
            col += profA[b]
        for b in g:
            colB[b] = col
            col += profB[b]
    S = int(col)

    plans = []
    for c in range(M_CORES):
        st = cores[c]
        # block position relabel: core's sorted block i -> profile position i
        pos_of = np.empty(NB, np.int64)
        pos_of[sorted_idx[c]] = np.arange(NB)
        blkpos = pos_of[st["e_blk"]]

        # lane assignment within (relabeled) block: order of appearance of dst
        assign_pos = pos_of[st["assign"]]          # local dst -> block position
        aorder = np.argsort(assign_pos, kind="stable")
        blk_sorted = assign_pos[aorder]
        starts = np.searchsorted(blk_sorted, np.arange(NB))
        lane = np.arange(DST_PAD) - starts[blk_sorted]
        rows = blk_sorted * 128 + lane
        perm = np.zeros(DST_PAD, np.int64)
        perm[rows] = aorder                        # device row -> local dst
        lane_of = np.zeros(DST_PAD, np.int64)
        lane_of[aorder] = lane

        idx_kv = np.zeros((128, S), np.int16)
        idx_q = np.zeros((128, S), np.int16)
        dstloc = np.full((128, S), 255.0, np.float32)

        # pack edges of (block position, class) into its chunk range
        key = blkpos * 2 + st["e_cls"]
        eorder = np.argsort(key, kind="stable")
        kb = key[eorder]
        counts = np.bincount(kb, minlength=NB * 2)
        estarts = np.concatenate([[0], np.cumsum(counts)[:-1]])
        j = np.arange(len(kb)) - estarts[kb]
        e_blkpos = kb // 2
        e_cls_s = kb % 2
        base_col = np.where(e_cls_s == 0, colA[e_blkpos], colB[e_blkpos])
        cap = np.where(e_cls_s == 0, profA[e_blkpos], profB[e_blkpos]) * 128
        if (j >= cap).any():
            raise RuntimeError("chunk overflow")
        scol = base_col + j // 128
        p_of = j % 128
        er = st["e_rank"][eorder]
        idx_kv[p_of, scol] = np.where(er < HALF, er, er - HALF).astype(np.int16)
        idx_q[p_of, scol] = (e_blkpos * 128
                             + lane_of[st["e_ld"][eorder]]).astype(np.int16)
        dstloc[p_of, scol] = lane_of[st["e_ld"][eorder]].astype(np.float32)

        plans.append(dict(idx16_kv=_wrap16(idx_kv), idx16_q=_wrap16(idx_q),
                          dstloc=dstloc, node_order=st["order"], perm=perm))
    return plans, profile


def _build_nc(profile, dst_pad=DST_PAD, tw=TW):
    from contextlib import ExitStack
    import concourse.bass as bass
    import concourse.tile as tile
    from concourse import bacc, mybir

    fp32 = mybir.dt.float32
    bf16 = mybir.dt.bfloat16
    i16 = mybir.dt.int16
    Alu = mybir.AluOpType
    Act = mybir.ActivationFunctionType

    nc = bacc.Bacc("TRN2", target_bir_lowering=False, debug=False)
    nb = len(profile)
    profA = [p[0] for p in profile]
    profB = [p[1] for p in profile]
    groups = []
    b0 = 0
    while b0 < nb:
        groups.append(tuple(range(b0, min(b0 + GB, nb))))
        b0 += GB
    colA = [0] * nb
    colB = [0] * nb
    col = 0
    for g in groups:
        for b in g:
            colA[b] = col
            col += profA[b]
        for b in g:
            colB[b] = col
            col += profB[b]
    S = int(col)

    xT = nc.dram_tensor("xT", [128, NREF], bf16, kind="ExternalInput").ap()
    xTs = nc.dram_tensor("xTs", [128, dst_pad], bf16, kind="ExternalInput").ap()
    Wq = nc.dram_tensor("Wq", [128, 128], bf16, kind="ExternalInput").ap()
    Wk = nc.dram_tensor("Wk", [128, 128], bf16, kind="ExternalInput").ap()
    Wv = nc.dram_tensor("Wv", [128, 128], bf16, kind="ExternalInput").ap()
    Ws = nc.dram_tensor("Ws", [128, 128], bf16, kind="ExternalInput").ap()
    bq1 = nc.dram_tensor("bq1", [1, 128], bf16, kind="ExternalInput").ap()
    bsv1 = nc.dram_tensor("bsv1", [1, 128], bf16, kind="ExternalInput").ap()
    ikv_d = nc.dram_tensor("idx16_kv", [128, S * 8], i16, kind="ExternalInput").ap()
    iq_d = nc.dram_tensor("idx16_q", [128, S * 8], i16, kind="ExternalInput").ap()
    dstloc_d = nc.dram_tensor("dstloc", [128, S], fp32, kind="ExternalInput").ap()

    kv_tab = nc.dram_tensor("kv_tab", [NREF, 256], bf16, kind="Internal").ap()
    q_tab = nc.dram_tensor("q_tab", [dst_pad, 128], bf16, kind="Internal").ap()
    out_d = nc.dram_tensor("out", [dst_pad, 128], fp32, kind="ExternalOutput").ap()

    with tile.TileContext(nc) as tc, ExitStack() as ctx:
        const_p = ctx.enter_context(tc.tile_pool(name="const", bufs=1))

        w_q = const_p.tile([128, 128], bf16, tag="wq")
        w_k = const_p.tile([128, 128], bf16, tag="wk")
        w_v = const_p.tile([128, 128], bf16, tag="wv")
        w_s = const_p.tile([128, 128], bf16, tag="ws")
        b_q = const_p.tile([1, 128], bf16, tag="bq")
        b_sv = const_p.tile([1, 128], bf16, tag="bsv")
        nc.sync.dma_start(w_q[:], Wq[:])
        nc.sync.dma_start(w_k[:], Wk[:])
        nc.sync.dma_start(w_v[:], Wv[:])
        nc.sync.dma_start(w_s[:], Ws[:])
        nc.sync.dma_start(b_q[:], bq1[:])
        nc.sync.dma_start(b_sv[:], bsv1[:])

        ones1 = const_p.tile([1, 128], bf16, tag="ones1")
        nc.vector.memset(ones1[:], 1.0)
        iota_i = const_p.tile([128, 128], mybir.dt.int32, tag="iota_i")
        nc.gpsimd.iota(iota_i[:], pattern=[[1, 128]], base=0, channel_multiplier=0)
        iota_f = const_p.tile([128, 128], fp32, tag="iota_f")
        nc.vector.tensor_copy(iota_f[:], iota_i[:])

        skip_sb = const_p.tile([128, nb, 128], fp32, tag="skip")
        ikv_sb = const_p.tile([128, S * 8], i16, tag="ikv")
        iq_sb = const_p.tile([128, S * 8], i16, tag="iq")
        dstloc_sb = const_p.tile([128, S], fp32, tag="dl")
        nc.sync.dma_start(ikv_sb[:], ikv_d[:])
        nc.sync.dma_start(iq_sb[:], iq_d[:])
        nc.sync.dma_start(dstloc_sb[:], dstloc_d[:])

        # ------------- phase 1b: q' (to DRAM) and skip for the dst slice ----
        q_stores = []
        n_full_b = dst_pad // tw
        tiles1b = [(i * tw, tw) for i in range(n_full_b)]
        if dst_pad % tw:
            tiles1b.append((n_full_b * tw, dst_pad % tw))
        with tc.tile_pool(name="p2x", bufs=3) as p2x, \
             tc.tile_pool(name="p2o", bufs=3) as p2o, \
             tc.tile_pool(name="p2ps", bufs=4, space="PSUM") as p2ps:
            for (base, w) in tiles1b:
                nj = w // 128
                xt = p2x.tile([128, w], bf16, tag="xst")
                nc.sync.dma_start(xt[:], xTs[:, base:base + w])
                qsb = p2o.tile([128, nj, 128], bf16, tag="qsb")
                for j in range(nj):
                    lhs = xt[:, j * 128:(j + 1) * 128]
                    blk = base // 128 + j
                    pq = p2ps.tile([128, 128], fp32, tag="ps2")
                    nc.tensor.matmul(out=pq[:], lhsT=lhs, rhs=w_q[:], start=True, stop=False)
                    nc.tensor.matmul(out=pq[:], lhsT=ones1[:], rhs=b_q[:], start=False, stop=True)
                    ps = p2ps.tile([128, 128], fp32, tag="ps2")
                    nc.tensor.matmul(out=ps[:], lhsT=lhs, rhs=w_s[:], start=True, stop=False)
                    nc.tensor.matmul(out=ps[:], lhsT=ones1[:], rhs=b_sv[:], start=False, stop=True)
                    nc.vector.tensor_copy(qsb[:, j, :], pq[:])
                    nc.scalar.activation(skip_sb[:, blk, :], ps[:], Act.Copy)
                out_view = q_tab[base:base + w, :].rearrange("(j p) e -> p j e", p=128)
                q_stores.append(nc.sync.dma_start(out_view, qsb[:]))

        # ---------------- phase 1a: v|k table for ranked nodes --------------
        kv_stores = []
        tiles1a = [(i * tw, tw) for i in range(NREF // tw)]
        with tc.tile_pool(name="p1x", bufs=4) as p1x, \
             tc.tile_pool(name="p1o", bufs=4) as p1o, \
             tc.tile_pool(name="p1ps", bufs=6, space="PSUM") as p1ps:
            for (base, w) in tiles1a:
                nj = w // 128
                xt = p1x.tile([128, w], bf16, tag="xt")
                nc.sync.dma_start(xt[:], xT[:, base:base + w])
                kvsb = p1o.tile([128, nj, 256], bf16, tag="kvsb")
                for j0 in range(0, nj, 4):
                    js = list(range(j0, min(j0 + 4, nj)))
                    g = len(js)
                    pk = p1ps.tile([128, g * 128], fp32, tag="ps")
                    pv = p1ps.tile([128, g * 128], fp32, tag="ps")
                    for i, j in enumerate(js):
                        lhs = xt[:, j * 128:(j + 1) * 128]
                        nc.tensor.matmul(out=pk[:, i * 128:(i + 1) * 128],
                                         lhsT=lhs, rhs=w_k[:], start=True, stop=True)
                        nc.tensor.matmul(out=pv[:, i * 128:(i + 1) * 128],
                                         lhsT=lhs, rhs=w_v[:], start=True, stop=True)
                    kv = kvsb[:, j0:j0 + g, :]
                    nc.vector.tensor_copy(kv[:, :, 0:128],
                                          pv[:].rearrange("p (c e) -> p c e", e=128))
                    nc.scalar.activation(kv[:, :, 128:256],
                                         pk[:].rearrange("p (c e) -> p c e", e=128),
                                         Act.Copy)
                out_view = kv_tab[base:base + w, :].rearrange("(j p) e -> p j e", p=128)
                kv_stores.append(nc.sync.dma_start(out_view, kvsb[:]))

        # ---------------- phase 2: edge attention + scatter ----------------
        from concourse.tile_rust import add_dep_helper
        first_kv = [None]
        first_q = [None]
        with tc.tile_pool(name="gka", bufs=3) as gka_p, \
             tc.tile_pool(name="gkb", bufs=3) as gkb_p, \
             tc.tile_pool(name="gq", bufs=3) as gq_p, \
             tc.tile_pool(name="prd", bufs=2) as prd_p, \
             tc.tile_pool(name="lgp", bufs=4) as lg_p, \
             tc.tile_pool(name="mex", bufs=10) as mex_p, \
             tc.tile_pool(name="epi", bufs=4) as epi_p, \
             tc.tile_pool(name="aps", bufs=4, space="PSUM") as aps_p:
            MAXC = 8   # <= 1024 indices per dma_gather (HW SWDGE limit)

            def gather_split(out_tile, in_ap, idx_sb, base_col, n_chunks, elem):
                insts = []
                for k0 in range(0, n_chunks, MAXC):
                    k1 = min(k0 + MAXC, n_chunks)
                    insts.append(nc.gpsimd.dma_gather(
                        out_ap=out_tile[:, k0:k1, :], in_ap=in_ap,
                        idxs_ap=idx_sb[:, (base_col + k0) * 8:(base_col + k1) * 8],
                        num_idxs=(k1 - k0) * 128, num_idxs_reg=(k1 - k0) * 128,
                        elem_size=elem))
                return insts

            for blocks in groups:
                nA = sum(profA[b] for b in blocks)
                nB_ = sum(profB[b] for b in blocks)
                CC = nA + nB_
                c0 = colA[blocks[0]]

                kvgA = gka_p.tile([128, nA, 256], bf16, tag="kvgA")
                gis = gather_split(kvgA, kv_tab[0:HALF, :], ikv_sb, c0, nA, 256)
                if first_kv[0] is None:
                    first_kv[0] = gis[0]
                    for s in kv_stores:
                        add_dep_helper(gis[0].ins, s.ins, reason="kv_tab raw")
                kvgB = None
                if nB_:
                    kvgB = gkb_p.tile([128, nB_, 256], bf16, tag="kvgB")
                    gather_split(kvgB, kv_tab[HALF:NREF, :], ikv_sb,
                                 c0 + nA, nB_, 256)
                qg = gq_p.tile([128, CC, 128], bf16, tag="qg")
                gqs = gather_split(qg, q_tab[:], iq_sb, c0, CC, 128)
                if first_q[0] is None:
                    first_q[0] = gqs[0]
                    for s in q_stores:
                        add_dep_helper(gqs[0].ins, s.ins, reason="q_tab raw")

                prodA = prd_p.tile([128, nA, 128], bf16, tag="prodA")
                nc.vector.tensor_tensor(
                    out=prodA[:], in0=qg[:, 0:nA, :],
                    in1=kvgA[:, :, 128:256], op=Alu.mult)
                lg = lg_p.tile([128, CC], fp32, tag="lg")
                nc.vector.reduce_sum(out=lg[:, 0:nA], in_=prodA[:],
                                     axis=mybir.AxisListType.X)
                if nB_:
                    prodB = prd_p.tile([128, nB_, 128], bf16, tag="prodB")
                    nc.vector.tensor_tensor(
                        out=prodB[:], in0=qg[:, nA:CC, :],
                        in1=kvgB[:, :, 128:256], op=Alu.mult)
                    nc.vector.reduce_sum(out=lg[:, nA:CC], in_=prodB[:],
                                         axis=mybir.AxisListType.X)
                exg = lg_p.tile([128, CC], fp32, tag="exg")
                nc.scalar.activation(exg[:], lg[:], Act.Exp)
                # den fold: overwrite k[0] with 1.0 after logits consumed k
                nc.vector.memset(kvgA[:, :, 128:129], 1.0)
                if nB_:
                    nc.vector.memset(kvgB[:, :, 128:129], 1.0)

                for b in blocks:
                    pagg = aps_p.tile([128, 129], fp32, tag="pagg")
                    ntot = profA[b] + profB[b]
                    done = 0
                    for (tile_, prof_b, coff) in ((kvgA, profA[b], colA[b] - c0),
                                                  (kvgB, profB[b], colB[b] - c0 - nA)):
                        for c in range(prof_b):
                            gcol = (colA[b] if tile_ is kvgA else colB[b]) + c
                            mex = mex_p.tile([128, 128], bf16, tag="mex")
                            nc.vector.tensor_scalar(
                                out=mex[:], in0=iota_f[:],
                                scalar1=dstloc_sb[:, gcol:gcol + 1],
                                scalar2=exg[:, gcol - c0:gcol - c0 + 1],
                                op0=Alu.is_equal, op1=Alu.mult)
                            nc.tensor.matmul(
                                out=pagg[:], lhsT=mex[:],
                                rhs=tile_[:, coff + c, 0:129],
                                start=(done == 0), stop=(done == ntot - 1))
                            done += 1
                    # epilogue: out = elu(agg/den + skip)
                    den = epi_p.tile([128, 1], fp32, tag="den")
                    nc.vector.tensor_scalar_add(den[:], pagg[:, 128:129], 1e-30)
                    rec = epi_p.tile([128, 1], fp32, tag="rec")
                    nc.vector.reciprocal(rec[:], den[:])
                    z2 = epi_p.tile([128, 128], fp32, tag="z2")
                    nc.vector.scalar_tensor_tensor(
                        out=z2[:], in0=pagg[:, 0:128], scalar=rec[:],
                        in1=skip_sb[:, b, :], op0=Alu.mult, op1=Alu.add)
                    zn = epi_p.tile([128, 128], fp32, tag="zn")
                    nc.vector.tensor_scalar_min(zn[:], z2[:], 0.0)
                    en = epi_p.tile([128, 128], fp32, tag="en")
                    nc.scalar.activation(en[:], zn[:], Act.Exp)
                    zp = epi_p.tile([128, 128], fp32, tag="zp")
                    nc.scalar.activation(zp[:], z2[:], Act.Relu)
                    o2 = epi_p.tile([128, 128], fp32, tag="o2")
                    nc.vector.scalar_tensor_tensor(
                        out=o2[:], in0=en[:], scalar=-1.0,
                        in1=zp[:], op0=Alu.add, op1=Alu.add)
                    nc.sync.dma_start(out_d[b * 128:(b + 1) * 128, :], o2[:])

    nc.compile()
    return nc


_NC_CACHE = {}


def _get_nc(profile):
    if profile not in _NC_CACHE:
        _NC_CACHE[profile] = _build_nc(profile)
    return _NC_CACHE[profile]


def _make_in_maps(inputs, plans):
    x = np.asarray(inputs["x"], np.float32)
    xb = x.astype(BF16)
    wq = (np.asarray(inputs["Wq"], np.float32) * SCALE).astype(BF16)
    wk = np.asarray(inputs["Wk"], np.float32).astype(BF16)
    wv = np.asarray(inputs["Wv"], np.float32).astype(BF16)
    ws = np.asarray(inputs["Ws"], np.float32).astype(BF16)
    bq1 = (np.asarray(inputs["bq"], np.float32) * SCALE).astype(BF16).reshape(1, 128)
    bsv1 = (np.asarray(inputs["bs"], np.float32)
            + np.asarray(inputs["bv"], np.float32)).astype(BF16).reshape(1, 128)

    in_maps = []
    for c in range(M_CORES):
        pl = plans[c]
        xT = np.zeros((128, NREF), BF16)
        sel = pl["node_order"][:NREF]
        xT[:, :len(sel)] = xb[sel].T
        xs_local = np.zeros((DST_PAD, 128), BF16)
        xs_local[:DPC] = xb[c * DPC:(c + 1) * DPC]
        xTs = xs_local[np.minimum(pl["perm"], DST_PAD - 1)].T.copy()
        in_maps.append({
            "xT": xT, "xTs": xTs,
            "Wq": wq, "Wk": wk, "Wv": wv, "Ws": ws,
            "bq1": bq1, "bsv1": bsv1,
            "idx16_kv": pl["idx16_kv"], "idx16_q": pl["idx16_q"],
            "dstloc": pl["dstloc"],
        })
    return in_maps


def kernel(x, edge_index, Wq, bq, Wk, bk, Wv, bv, Ws, bs):
    from concourse import bass_utils

    plans, profile = _host_prep(edge_index)
    in_maps = _make_in_maps(
        {"x": x, "Wq": Wq, "Wk": Wk, "Wv": Wv, "Ws": Ws,
         "bq": bq, "bs": bs, "bv": bv}, plans)
    nc = _get_nc(profile)
    res = bass_utils.run_bass_kernel_spmd(nc, in_maps, core_ids=list(range(M_CORES)))
    out = np.zeros((N, 128), np.float32)
    for c in range(M_CORES):
        rows = res.results[c]["out"]          # [DST_PAD, 128] in device order
        p = plans[c]["perm"]
        valid = p < DPC
        out[c * DPC + p[valid]] = rows[valid]
    return out
